# revision 30
# baseline (speedup 1.0000x reference)
"""Trainium2 Bass kernel for BinaryTokenClassificationModel (segment_reduce).

Reference semantics (B=16, L=2048, H=1024, W=1024):
    src = segment_mean(hidden, source_word_ids)   # [B,W,H]
    tgt = segment_mean(hidden, target_word_ids)   # [B,W,H]
    logits[b,s,t,0] = src[b,s]@w_s + tgt[b,t]@w_t + bias

Because the classifier is linear, the pooled [B,W,H] tensors are never
materialized:
    src_proj[b,s] = segment_mean_s( hidden[b,l] @ w_s )
so the per-token scalar dots are computed on the PE (hidden is
pre-transposed to [H,L] on the host so it streams through the PE as the
moving operand against a tiny stationary [128,2] weight tile), the
scalar dots are segment-reduced via one-hot matmuls on the PE (word =
128*q + r factorization), and the [W,W] output is an outer
broadcast-sum emitted in bf16 (upcast on the host).

The two batches per core are software-pipelined: batch 0's epilogue
(transposes, segment matmuls, broadcast, output adds) is emitted inside
batch 1's DMA-paced dot stream so the input DMA never stalls.

Sharding: data-parallel over batch — 2 examples per NeuronCore on 8
cores; classifier weights replicated.
"""

from contextlib import ExitStack

import ml_dtypes
import numpy as np

import concourse.mybir as mybir
import concourse.tile as tile
from concourse import bacc
from concourse.bass_utils import run_bass_kernel_spmd
from concourse.masks import make_identity

P = 128          # partitions
B = 16           # full batch
NCORES = 8
BLOC = B // NCORES   # batches per core = 2
L = 2048         # tokens
H = 1024         # hidden
W = 1024         # words
Q = W // P       # 8 word chunks
NI = L // P      # 16 token tiles per batch (token l = p*NI + i)
NH = H // P      # 8 hidden chunks
NCH = 4          # 512-token psum chunks for the dot matmuls
CHW = L // NCH   # 512

F32 = mybir.dt.float32
BF16 = mybir.dt.bfloat16
I32 = mybir.dt.int32

_CACHE = {}


def _build_module():
    nc = bacc.Bacc(None, target_bir_lowering=False, debug=False)
    names = {}
    with tile.TileContext(nc) as tc, ExitStack() as ctx:
        dram = ctx.enter_context(tc.tile_pool(name="dram", bufs=1, space="DRAM"))
        sb_c = ctx.enter_context(tc.tile_pool(name="const", bufs=1))
        sb_h = ctx.enter_context(tc.tile_pool(name="hid", bufs=8))
        sb_s = ctx.enter_context(tc.tile_pool(name="small", bufs=2))
        sb_o = ctx.enter_context(tc.tile_pool(name="outp", bufs=6))
        ps_d = ctx.enter_context(tc.tile_pool(name="psdot", bufs=1, space="PSUM"))
        ps = ctx.enter_context(tc.tile_pool(name="psum", bufs=1, space="PSUM"))

        # host-packed hidden^T: row-pair layout [t*128+p, k*L+l] =
        # hidT[t*256 + k*128 + p, l] so a pair tile is one contiguous
        # 8KB run per partition (half the DMA descriptors).
        hidT_d = [dram.tile([H // 2, 2 * L], BF16, kind="ExternalInput",
                            name=f"hidT{b}") for b in range(BLOC)]
        src_d = [dram.tile([L], I32, kind="ExternalInput", name=f"srcids{b}")
                 for b in range(BLOC)]
        tgt_d = [dram.tile([L], I32, kind="ExternalInput", name=f"tgtids{b}")
                 for b in range(BLOC)]
        w_d = dram.tile([P, NH, 2], BF16, kind="ExternalInput")  # host-swizzled
        b_d = dram.tile([P, 1], F32, kind="ExternalInput")
        out_d = [dram.tile([W, W], BF16, kind="ExternalOutput", name=f"logits{b}")
                 for b in range(BLOC)]

        names["hidT"] = [t.name for t in hidT_d]
        names["src"] = [t.name for t in src_d]
        names["tgt"] = [t.name for t in tgt_d]
        names["w"] = w_d.name
        names["b"] = b_d.name
        names["out"] = [t.name for t in out_d]

        # ---- constants ----
        # iota_r16[p, i, r] = r ; iota_q16[p, i, q] = q  (bf16: ints < 256 exact)
        iota_r16 = sb_c.tile([P, NI, P], BF16, tag="ior")
        nc.gpsimd.iota(iota_r16[:], pattern=[[0, NI], [1, P]], base=0,
                       channel_multiplier=0, allow_small_or_imprecise_dtypes=True)
        iota_q16 = sb_c.tile([P, NI, Q], BF16, tag="ioq")
        nc.gpsimd.iota(iota_q16[:], pattern=[[0, NI], [1, Q]], base=0,
                       channel_multiplier=0, allow_small_or_imprecise_dtypes=True)
        ident = sb_c.tile([P, P], BF16, tag="id")
        make_identity(nc, ident[:])
        ones = sb_c.tile([P, P], BF16, tag="ones")
        nc.vector.memset(ones[:], 1.0)
        ident2 = sb_c.tile([2, 2], F32, tag="id2")
        make_identity(nc, ident2[:])

        w2 = sb_c.tile([P, NH, 2], BF16, tag="w2")
        nc.scalar.dma_start(out=w2[:], in_=w_d[:])
        b_bc = sb_c.tile([P, 1], F32, tag="bb")
        nc.scalar.dma_start(out=b_bc[:], in_=b_d[:])

        # ---------------- stage builders ----------------

        def build_onehots(b):
            """ids -> (q, r) one-hots; ids math on Pool, is_eq split DVE/Pool."""
            oralls = {}
            mdoq = {}
            for side, ids_dram, eng in (("s", src_d[b], nc.vector),
                                        ("t", tgt_d[b], nc.vector)):
                ids_t = sb_s.tile([P, NI], I32, tag=f"ids{side}")
                nc.sync.dma_start(out=ids_t[:],
                                  in_=ids_dram[:].rearrange("(p i) -> p i", p=P))
                q_i = sb_s.tile([P, NI], I32, tag=f"qi{side}")
                r_i = sb_s.tile([P, NI], I32, tag=f"ri{side}")
                nc.vector.tensor_scalar(out=q_i[:], in0=ids_t[:], scalar1=7,
                                        scalar2=None,
                                        op0=mybir.AluOpType.logical_shift_right)
                nc.vector.tensor_scalar(out=r_i[:], in0=ids_t[:], scalar1=127,
                                        scalar2=None,
                                        op0=mybir.AluOpType.bitwise_and)
                qf = sb_s.tile([P, NI], BF16, tag=f"qf{side}")
                rf = sb_s.tile([P, NI], BF16, tag=f"rf{side}")
                nc.vector.tensor_copy(out=qf[:], in_=q_i[:])
                nc.vector.tensor_copy(out=rf[:], in_=r_i[:])
                oralls[side] = sb_s.tile([P, NI, P], BF16, tag=f"orall{side}",
                                         name=f"orall{side}")
                eng.tensor_tensor(
                    out=oralls[side][:], in0=iota_r16[:],
                    in1=rf[:].to_broadcast([P, NI, P]),
                    op=mybir.AluOpType.is_equal)
                mdoq[side] = sb_s.tile([P, NI, 2 * Q], BF16, tag=f"mdoq{side}",
                                       name=f"mdoq{side}")
                eng.tensor_tensor(
                    out=mdoq[side][:, :, Q:2 * Q], in0=iota_q16[:],
                    in1=qf[:].to_broadcast([P, NI, Q]),
                    op=mybir.AluOpType.is_equal)
            return oralls, mdoq

        def dots_chunk(b, pdots, c):
            """Chunk c of hidden^T through the PE.  c=0,1: single h-chunks
            (fast pipeline start); c=2,3,4: row-pair tiles (8KB runs)."""
            if c < 2:
                hq = c
                ht = sb_h.tile([P, L], BF16, tag="ht1", bufs=3)
                nc.sync.dma_start(out=ht[:], in_=hidT_d[b][0:P,
                                                           hq * L:(hq + 1) * L])
                for n in range(NCH):
                    nc.tensor.matmul(out=pdots[n][:], lhsT=w2[:, hq, :],
                                     rhs=ht[:, n * CHW:(n + 1) * CHW],
                                     start=(hq == 0), stop=False)
                return
            t = c - 1
            ht2 = sb_h.tile([P, 2, L], BF16, tag="ht2", bufs=3)
            nc.sync.dma_start(out=ht2[:], in_=hidT_d[b][t * P:(t + 1) * P, :])
            for k in range(2):
                hq = 2 * t + k
                for n in range(NCH):
                    nc.tensor.matmul(out=pdots[n][:], lhsT=w2[:, hq, :],
                                     rhs=ht2[:, k, n * CHW:(n + 1) * CHW],
                                     start=False, stop=(hq == NH - 1))

        def dance(b, pdots):
            """psum dot chunks -> token-partition layout psT[p, i, side]."""
            dots_sb = sb_s.tile([2, P, NI], F32, tag="dots", name="dots")
            for n in range(NCH):
                dst = dots_sb[:, n * (P // NCH):(n + 1) * (P // NCH), :]
                if n % 2 == 0:
                    nc.scalar.copy(out=dst, in_=pdots[n][:])
                else:
                    nc.vector.tensor_copy(out=dst, in_=pdots[n][:])
            psT = ps.tile([P, NI, 2], F32, space="PSUM", tag="psT", name="psT",
                          bufs=1)
            for i in range(NI):
                nc.tensor.transpose(out=psT[:, i, :], in_=dots_sb[:, :, i],
                                    identity=ident2[:])
            return psT

        def seg_reduce(b, psT, oralls, mdoq):
            """md = onehot_q * dots; segment sums+counts via PE matmuls.
            Side t first: it gates the long broadcast chain."""
            segP = ps.tile([P, 2, 2 * Q], F32, space="PSUM", tag="segP",
                           name="segP", bufs=1)
            for sidx, side in ((1, "t"), (0, "s")):
                nc.vector.tensor_tensor(
                    out=mdoq[side][:, :, 0:Q], in0=mdoq[side][:, :, Q:2 * Q],
                    in1=psT[:, :, sidx:sidx + 1].to_broadcast([P, NI, Q]),
                    op=mybir.AluOpType.mult)
                for i in range(NI):
                    nc.tensor.matmul(out=segP[:, sidx, :],
                                     lhsT=oralls[side][:, i, :],
                                     rhs=mdoq[side][:, i, :],
                                     start=(i == 0), stop=(i == NI - 1))
            return segP

        def _proj_side(segP, sidx):
            cnt = sb_s.tile([P, Q], F32, tag="cnt")
            nc.vector.tensor_scalar(out=cnt[:], in0=segP[:, sidx, Q:2 * Q],
                                    scalar1=1.0, scalar2=None,
                                    op0=mybir.AluOpType.max)
            rec = sb_s.tile([P, Q], F32, tag="rec")
            nc.vector.reciprocal(out=rec[:], in_=cnt[:])
            pr = sb_s.tile([P, Q], F32, tag=f"proj{sidx}", name=f"proj{sidx}")
            nc.vector.tensor_tensor(out=pr[:], in0=segP[:, sidx, 0:Q],
                                    in1=rec[:], op=mybir.AluOpType.mult)
            return pr

        def epilogue(b, segP):
            """divide sums by counts; build [P, W] broadcast of tgt proj."""
            # target side first: msel/bc is the long pole
            proj_t = _proj_side(segP, 1)
            msel = sb_s.tile([P, W], BF16, tag="msel")
            for qb in range(Q):
                dst = msel[:, qb * P:(qb + 1) * P]
                if qb % 2 == 0:
                    nc.scalar.mul(out=dst, in_=ident[:],
                                  mul=proj_t[:, qb:qb + 1])
                else:
                    nc.vector.tensor_scalar(
                        out=dst, in0=ident[:],
                        scalar1=proj_t[:, qb:qb + 1], scalar2=None,
                        op0=mybir.AluOpType.mult)
            bc_sb = sb_s.tile([P, W], BF16, tag="bcsb")
            for half in range(2):
                bc_ps = ps.tile([P, W // 2], F32, space="PSUM", tag="bc",
                                bufs=2)
                nc.tensor.matmul(out=bc_ps[:], lhsT=ones[:],
                                 rhs=msel[:, half * (W // 2):(half + 1) * (W // 2)],
                                 start=True, stop=True)
                nc.scalar.copy(out=bc_sb[:, half * (W // 2):(half + 1) * (W // 2)],
                               in_=bc_ps[:])

            proj_s = _proj_side(segP, 0)
            proj_sb = sb_s.tile([P, Q], F32, tag="projsb")
            nc.vector.tensor_scalar(out=proj_sb[:], in0=proj_s[:],
                                    scalar1=b_bc[:, 0:1], scalar2=None,
                                    op0=mybir.AluOpType.add)
            return proj_sb, bc_sb

        def outputs(b, proj_sb, bc_sb, final=False):
            """out[j*128+p, t] = proj_s[p, j] + tp[t]; adds on DVE.
            DMA triggers cost ~600ns of issuing-engine queue time, so they
            go on the idle Pool queue (mid-stream batch, j-pair tiles).
            The final batch's drain is the kernel tail: single-j tiles
            alternating Pool/Sync rings for 2x drain parallelism."""
            out_ap = out_d[b][:].rearrange("(p j) t -> p j t", p=P)
            if final:
                for j in range(Q):
                    ot = sb_o.tile([P, W], BF16, tag="otf")
                    nc.vector.tensor_scalar(
                        out=ot[:], in0=bc_sb[:],
                        scalar1=proj_sb[:, j:j + 1],
                        scalar2=None, op0=mybir.AluOpType.add)
                    eng = nc.gpsimd if j % 2 == 0 else nc.sync
                    eng.dma_start(out=out_ap[:, j, :], in_=ot[:])
                return
            for jp in range(Q // 2):
                ot = sb_o.tile([P, 2, W], BF16, tag="ot")
                for k in range(2):
                    j = 2 * jp + k
                    nc.vector.tensor_scalar(
                        out=ot[:, k, :], in0=bc_sb[:],
                        scalar1=proj_sb[:, j:j + 1],
                        scalar2=None, op0=mybir.AluOpType.add)
                nc.gpsimd.dma_start(out=out_ap[:, 2 * jp:2 * jp + 2, :],
                                    in_=ot[:])

        # ---------------- pipelined emission ----------------
        # first hidden DMAs lead the sync queue so the input stream starts
        # at t=0; one-hot builds (DVE) ride alongside.
        pd0 = [ps_d.tile([2, P // NCH, NI], F32, space="PSUM", tag=f"pd{n}",
                         name=f"pd{n}") for n in range(NCH)]
        dots_chunk(0, pd0, 0)
        dots_chunk(0, pd0, 1)
        oh0 = build_onehots(0)
        for c in range(2, 5):
            dots_chunk(0, pd0, c)
        oh1 = build_onehots(1)
        psT0 = dance(0, pd0)

        # batch 1 dots start immediately (psum chunks freed by the dance
        # copies); batch 0's epilogue PE work rides in the DMA-paced gaps.
        pd1 = [ps_d.tile([2, P // NCH, NI], F32, space="PSUM", tag=f"pd{n}",
                         name=f"pd{n}") for n in range(NCH)]
        dots_chunk(1, pd1, 0)

        segP0 = seg_reduce(0, psT0, *oh0)
        proj0, bc0 = epilogue(0, segP0)

        for c in range(1, 5):
            dots_chunk(1, pd1, c)

        outputs(0, proj0, bc0)

        psT1 = dance(1, pd1)
        segP1 = seg_reduce(1, psT1, *oh1)
        proj1, bc1 = epilogue(1, segP1)
        outputs(1, proj1, bc1, final=True)

    nc.compile()
    return nc, names


def _get_module():
    if "mod" not in _CACHE:
        _CACHE["mod"] = _build_module()
    return _CACHE["mod"]


def _run(hidden, classifier_w, classifier_b, source_word_ids, target_word_ids,
         **spmd_kwargs):
    nc, names = _get_module()
    bf16 = ml_dtypes.bfloat16
    # [B, H, L], then pack row pairs so each pair tile is one contiguous
    # 8KB-per-partition DMA run: packed[b, t*128+p, k*L+l] = hidT[b, t*256+k*128+p, l]
    hidT = np.asarray(hidden).astype(bf16).transpose(0, 2, 1)
    hidT = np.ascontiguousarray(
        hidT.reshape(B, NH // 2, 2, P, L).transpose(0, 1, 3, 2, 4)
        .reshape(B, H // 2, 2 * L))
    w = np.asarray(classifier_w, dtype=np.float32).reshape(2 * H)
    # w2[hp, hq, s] = w_side[s][hq*128 + hp]  (device layout, no gather DMA)
    w2 = np.ascontiguousarray(
        np.stack([w[:H], w[H:]], axis=-1).reshape(NH, P, 2)
        .transpose(1, 0, 2).astype(bf16))
    bias = np.ascontiguousarray(
        np.broadcast_to(np.asarray(classifier_b, dtype=np.float32)
                        .reshape(1, 1), (P, 1)))
    src = np.ascontiguousarray(source_word_ids, dtype=np.int32)
    tgt = np.ascontiguousarray(target_word_ids, dtype=np.int32)

    in_maps = []
    for c in range(NCORES):
        m = {names["w"]: w2, names["b"]: bias}
        for b in range(BLOC):
            gb = c * BLOC + b
            m[names["hidT"][b]] = hidT[gb]
            m[names["src"][b]] = src[gb]
            m[names["tgt"][b]] = tgt[gb]
        in_maps.append(m)

    res = run_bass_kernel_spmd(nc, in_maps, core_ids=list(range(NCORES)),
                               **spmd_kwargs)
    out = np.empty((B, W, W, 1), dtype=np.float32)
    for c in range(NCORES):
        for b in range(BLOC):
            # device rows are (p, j)-major; un-permute during the upcast
            out[c * BLOC + b, :, :, 0] = (
                res.results[c][names["out"][b]].reshape(P, Q, W)
                .transpose(1, 0, 2).reshape(W, W).astype(np.float32))
    return out, res


def kernel(hidden, classifier_w, classifier_b, source_word_ids,
           target_word_ids, num_words):
    out, _ = _run(hidden, classifier_w, classifier_b, source_word_ids,
                  target_word_ids)
    return out


# revision 34
# speedup vs baseline: 1.0426x; 1.0426x over previous
"""Trainium2 Bass kernel for BinaryTokenClassificationModel (segment_reduce).

Reference semantics (B=16, L=2048, H=1024, W=1024):
    src = segment_mean(hidden, source_word_ids)   # [B,W,H]
    tgt = segment_mean(hidden, target_word_ids)   # [B,W,H]
    logits[b,s,t,0] = src[b,s]@w_s + tgt[b,t]@w_t + bias

Because the classifier is linear, the pooled [B,W,H] tensors are never
materialized:
    src_proj[b,s] = segment_mean_s( hidden[b,l] @ w_s )
so the per-token scalar dots are computed on the PE (hidden is
pre-transposed to [H,L] on the host so it streams through the PE as the
moving operand against a tiny stationary [128,2] weight tile), the
scalar dots are segment-reduced via one-hot matmuls on the PE (word =
128*q + r factorization), and the [W,W] output is an outer
broadcast-sum emitted in bf16 (upcast on the host).

The two batches per core are software-pipelined: batch 0's epilogue
(transposes, segment matmuls, broadcast, output adds) is emitted inside
batch 1's DMA-paced dot stream so the input DMA never stalls.

Sharding: data-parallel over batch — 2 examples per NeuronCore on 8
cores; classifier weights replicated.
"""

from contextlib import ExitStack

import ml_dtypes
import numpy as np

import concourse.mybir as mybir
import concourse.tile as tile
from concourse import bacc
from concourse.bass_utils import run_bass_kernel_spmd
from concourse.masks import make_identity

P = 128          # partitions
B = 16           # full batch
NCORES = 8
BLOC = B // NCORES   # batches per core = 2
L = 2048         # tokens
H = 1024         # hidden
W = 1024         # words
Q = W // P       # 8 word chunks
NI = L // P      # 16 token tiles per batch (token l = p*NI + i)
NH = H // P      # 8 hidden chunks
NCH = 4          # 512-token psum chunks for the dot matmuls
CHW = L // NCH   # 512

F32 = mybir.dt.float32
BF16 = mybir.dt.bfloat16
I32 = mybir.dt.int32

_CACHE = {}


def _build_module():
    nc = bacc.Bacc(None, target_bir_lowering=False, debug=False)
    names = {}
    with tile.TileContext(nc) as tc, ExitStack() as ctx:
        dram = ctx.enter_context(tc.tile_pool(name="dram", bufs=1, space="DRAM"))
        sb_c = ctx.enter_context(tc.tile_pool(name="const", bufs=1))
        sb_h = ctx.enter_context(tc.tile_pool(name="hid", bufs=8))
        sb_s = ctx.enter_context(tc.tile_pool(name="small", bufs=2))
        sb_o = ctx.enter_context(tc.tile_pool(name="outp", bufs=6))
        ps_d = ctx.enter_context(tc.tile_pool(name="psdot", bufs=1, space="PSUM"))
        ps = ctx.enter_context(tc.tile_pool(name="psum", bufs=1, space="PSUM"))

        # host-packed hidden^T: row-pair layout [t*128+p, k*L+l] =
        # hidT[t*256 + k*128 + p, l] so a pair tile is one contiguous
        # 8KB run per partition (half the DMA descriptors).
        hidT_d = [dram.tile([H // 2, 2 * L], BF16, kind="ExternalInput",
                            name=f"hidT{b}") for b in range(BLOC)]
        src_d = [dram.tile([L], I32, kind="ExternalInput", name=f"srcids{b}")
                 for b in range(BLOC)]
        tgt_d = [dram.tile([L], I32, kind="ExternalInput", name=f"tgtids{b}")
                 for b in range(BLOC)]
        w_d = dram.tile([P, NH, 2], BF16, kind="ExternalInput")  # host-swizzled
        b_d = dram.tile([P, 1], F32, kind="ExternalInput")
        out_d = [dram.tile([W, W], BF16, kind="ExternalOutput", name=f"logits{b}")
                 for b in range(BLOC)]

        names["hidT"] = [t.name for t in hidT_d]
        names["src"] = [t.name for t in src_d]
        names["tgt"] = [t.name for t in tgt_d]
        names["w"] = w_d.name
        names["b"] = b_d.name
        names["out"] = [t.name for t in out_d]

        # ---- constants ----
        # iota_r16[p, i, r] = r ; iota_q16[p, i, q] = q  (bf16: ints < 256 exact)
        iota_r16 = sb_c.tile([P, NI, P], BF16, tag="ior")
        nc.gpsimd.iota(iota_r16[:], pattern=[[0, NI], [1, P]], base=0,
                       channel_multiplier=0, allow_small_or_imprecise_dtypes=True)
        iota_q16 = sb_c.tile([P, NI, Q], BF16, tag="ioq")
        nc.gpsimd.iota(iota_q16[:], pattern=[[0, NI], [1, Q]], base=0,
                       channel_multiplier=0, allow_small_or_imprecise_dtypes=True)
        ident = sb_c.tile([P, P], BF16, tag="id")
        make_identity(nc, ident[:])
        ones = sb_c.tile([P, P], BF16, tag="ones")
        nc.vector.memset(ones[:], 1.0)
        ident2 = sb_c.tile([2, 2], F32, tag="id2")
        make_identity(nc, ident2[:])

        w2 = sb_c.tile([P, NH, 2], BF16, tag="w2")
        nc.scalar.dma_start(out=w2[:], in_=w_d[:])
        b_bc = sb_c.tile([P, 1], F32, tag="bb")
        nc.scalar.dma_start(out=b_bc[:], in_=b_d[:])

        # ---------------- stage builders ----------------

        def build_onehots(b):
            """ids -> (q, r) one-hots; ids math on Pool, is_eq split DVE/Pool."""
            oralls = {}
            mdoq = {}
            for side, ids_dram, eng in (("s", src_d[b], nc.vector),
                                        ("t", tgt_d[b], nc.vector)):
                ids_t = sb_s.tile([P, NI], I32, tag=f"ids{side}")
                nc.sync.dma_start(out=ids_t[:],
                                  in_=ids_dram[:].rearrange("(p i) -> p i", p=P))
                q_i = sb_s.tile([P, NI], I32, tag=f"qi{side}")
                r_i = sb_s.tile([P, NI], I32, tag=f"ri{side}")
                nc.vector.tensor_scalar(out=q_i[:], in0=ids_t[:], scalar1=7,
                                        scalar2=None,
                                        op0=mybir.AluOpType.logical_shift_right)
                nc.vector.tensor_scalar(out=r_i[:], in0=ids_t[:], scalar1=127,
                                        scalar2=None,
                                        op0=mybir.AluOpType.bitwise_and)
                qf = sb_s.tile([P, NI], BF16, tag=f"qf{side}")
                rf = sb_s.tile([P, NI], BF16, tag=f"rf{side}")
                nc.vector.tensor_copy(out=qf[:], in_=q_i[:])
                nc.vector.tensor_copy(out=rf[:], in_=r_i[:])
                oralls[side] = sb_s.tile([P, NI, P], BF16, tag=f"orall{side}",
                                         name=f"orall{side}")
                eng.tensor_tensor(
                    out=oralls[side][:], in0=iota_r16[:],
                    in1=rf[:].to_broadcast([P, NI, P]),
                    op=mybir.AluOpType.is_equal)
                mdoq[side] = sb_s.tile([P, NI, 2 * Q], BF16, tag=f"mdoq{side}",
                                       name=f"mdoq{side}")
                eng.tensor_tensor(
                    out=mdoq[side][:, :, Q:2 * Q], in0=iota_q16[:],
                    in1=qf[:].to_broadcast([P, NI, Q]),
                    op=mybir.AluOpType.is_equal)
            return oralls, mdoq

        def dots_chunk(b, pdots, c):
            """Chunk c of hidden^T through the PE.  c=0,1: single h-chunks
            (fast pipeline start); c=2,3,4: row-pair tiles (8KB runs)."""
            if c < 2:
                hq = c
                ht = sb_h.tile([P, L], BF16, tag="ht1", bufs=3)
                nc.sync.dma_start(out=ht[:], in_=hidT_d[b][0:P,
                                                           hq * L:(hq + 1) * L])
                for n in range(NCH):
                    nc.tensor.matmul(out=pdots[n][:], lhsT=w2[:, hq, :],
                                     rhs=ht[:, n * CHW:(n + 1) * CHW],
                                     start=(hq == 0), stop=False)
                return
            t = c - 1
            ht2 = sb_h.tile([P, 2, L], BF16, tag="ht2", bufs=3)
            nc.sync.dma_start(out=ht2[:], in_=hidT_d[b][t * P:(t + 1) * P, :])
            for k in range(2):
                hq = 2 * t + k
                for n in range(NCH):
                    nc.tensor.matmul(out=pdots[n][:], lhsT=w2[:, hq, :],
                                     rhs=ht2[:, k, n * CHW:(n + 1) * CHW],
                                     start=False, stop=(hq == NH - 1))

        def dance(b, pdots):
            """psum dot chunks -> token-partition layout psT[p, i, side]."""
            dots_sb = sb_s.tile([2, P, NI], F32, tag="dots", name="dots")
            for n in range(NCH):
                dst = dots_sb[:, n * (P // NCH):(n + 1) * (P // NCH), :]
                if n % 2 == 0:
                    nc.scalar.copy(out=dst, in_=pdots[n][:])
                else:
                    nc.vector.tensor_copy(out=dst, in_=pdots[n][:])
            psT = ps.tile([P, NI, 2], F32, space="PSUM", tag="psT", name="psT",
                          bufs=1)
            for i in range(NI):
                nc.tensor.transpose(out=psT[:, i, :], in_=dots_sb[:, :, i],
                                    identity=ident2[:])
            return psT

        def seg_counts(b, oralls, mdoq):
            """Segment counts depend only on the one-hots: accumulate them
            (and 1/max(cnt,1)) mid-stream, off the critical tail chain."""
            # counts in region [:, :, 0, :]; sums land later in [:, :, 1, :]
            seg = ps.tile([P, 2, 2, Q], F32, space="PSUM", tag="seg",
                          name="seg", bufs=1)
            for sidx, side in ((1, "t"), (0, "s")):
                for i in range(NI):
                    nc.tensor.matmul(out=seg[:, sidx, 0, :],
                                     lhsT=oralls[side][:, i, :],
                                     rhs=mdoq[side][:, i, Q:2 * Q],
                                     start=(i == 0), stop=(i == NI - 1))
            cnt = sb_s.tile([P, 2, Q], F32, tag="cnt")
            nc.vector.tensor_scalar(out=cnt[:], in0=seg[:, :, 0, :],
                                    scalar1=1.0,
                                    scalar2=None, op0=mybir.AluOpType.max)
            rec = sb_s.tile([P, 2, Q], F32, tag="rec")
            nc.vector.reciprocal(out=rec[:], in_=cnt[:])
            return seg, rec

        def seg_reduce(b, seg, psT, oralls, mdoq):
            """md = onehot_q * dots; segment sums via PE matmuls.
            Side t first: it gates the long broadcast chain."""
            for sidx, side in ((1, "t"), (0, "s")):
                nc.vector.tensor_tensor(
                    out=mdoq[side][:, :, 0:Q], in0=mdoq[side][:, :, Q:2 * Q],
                    in1=psT[:, :, sidx:sidx + 1].to_broadcast([P, NI, Q]),
                    op=mybir.AluOpType.mult)
                for i in range(NI):
                    nc.tensor.matmul(out=seg[:, sidx, 1, :],
                                     lhsT=oralls[side][:, i, :],
                                     rhs=mdoq[side][:, i, 0:Q],
                                     start=(i == 0), stop=(i == NI - 1))
            return seg

        def _proj_side(seg, rec, sidx):
            pr = sb_s.tile([P, Q], F32, tag=f"proj{sidx}", name=f"proj{sidx}")
            nc.vector.tensor_tensor(out=pr[:], in0=seg[:, sidx, 1, :],
                                    in1=rec[:, sidx, :],
                                    op=mybir.AluOpType.mult)
            return pr

        def epilogue(b, seg, rec):
            """divide sums by precomputed reciprocals; build [P, W]
            broadcast of tgt proj."""
            # target side first: msel/bc is the long pole
            proj_t = _proj_side(seg, rec, 1)
            msel = sb_s.tile([P, W], BF16, tag="msel")
            for qb in range(Q):
                dst = msel[:, qb * P:(qb + 1) * P]
                if qb % 2 == 0:
                    nc.scalar.mul(out=dst, in_=ident[:],
                                  mul=proj_t[:, qb:qb + 1])
                else:
                    nc.vector.tensor_scalar(
                        out=dst, in0=ident[:],
                        scalar1=proj_t[:, qb:qb + 1], scalar2=None,
                        op0=mybir.AluOpType.mult)
            bc_sb = sb_s.tile([P, W], BF16, tag="bcsb")
            for half in range(2):
                bc_ps = ps.tile([P, W // 2], F32, space="PSUM", tag="bc",
                                bufs=2)
                nc.tensor.matmul(out=bc_ps[:], lhsT=ones[:],
                                 rhs=msel[:, half * (W // 2):(half + 1) * (W // 2)],
                                 start=True, stop=True)
                nc.scalar.copy(out=bc_sb[:, half * (W // 2):(half + 1) * (W // 2)],
                               in_=bc_ps[:])

            proj_s = _proj_side(seg, rec, 0)
            proj_sb = sb_s.tile([P, Q], F32, tag="projsb")
            nc.vector.tensor_scalar(out=proj_sb[:], in0=proj_s[:],
                                    scalar1=b_bc[:, 0:1], scalar2=None,
                                    op0=mybir.AluOpType.add)
            return proj_sb, bc_sb

        def outputs(b, proj_sb, bc_sb, final=False):
            """out[j*128+p, t] = proj_s[p, j] + tp[t]; adds on DVE.
            DMA triggers cost ~600ns of issuing-engine queue time, so they
            go on the idle Pool queue (mid-stream batch, j-pair tiles).
            The final batch's drain is the kernel tail: single-j tiles
            alternating Pool/Sync rings for 2x drain parallelism."""
            out_ap = out_d[b][:].rearrange("(p j) t -> p j t", p=P)
            if final:
                for j in range(Q):
                    ot = sb_o.tile([P, W], BF16, tag="otf")
                    nc.vector.tensor_scalar(
                        out=ot[:], in0=bc_sb[:],
                        scalar1=proj_sb[:, j:j + 1],
                        scalar2=None, op0=mybir.AluOpType.add)
                    eng = (nc.gpsimd, nc.sync, nc.scalar)[j % 3]
                    eng.dma_start(out=out_ap[:, j, :], in_=ot[:])
                return
            for jp in range(Q // 2):
                ot = sb_o.tile([P, 2, W], BF16, tag="ot")
                for k in range(2):
                    j = 2 * jp + k
                    nc.vector.tensor_scalar(
                        out=ot[:, k, :], in0=bc_sb[:],
                        scalar1=proj_sb[:, j:j + 1],
                        scalar2=None, op0=mybir.AluOpType.add)
                nc.gpsimd.dma_start(out=out_ap[:, 2 * jp:2 * jp + 2, :],
                                    in_=ot[:])

        # ---------------- pipelined emission ----------------
        # first hidden DMAs lead the sync queue so the input stream starts
        # at t=0; one-hot builds (DVE) ride alongside.
        pd0 = [ps_d.tile([2, P // NCH, NI], F32, space="PSUM", tag=f"pd{n}",
                         name=f"pd{n}") for n in range(NCH)]
        dots_chunk(0, pd0, 0)
        dots_chunk(0, pd0, 1)
        oh0 = build_onehots(0)
        for c in range(2, 5):
            dots_chunk(0, pd0, c)
        oh1 = build_onehots(1)
        seg0, rec0 = seg_counts(0, *oh0)
        seg1, rec1 = seg_counts(1, *oh1)
        psT0 = dance(0, pd0)

        # batch 1 dots start immediately (psum chunks freed by the dance
        # copies); batch 0's epilogue PE work rides in the DMA-paced gaps.
        pd1 = [ps_d.tile([2, P // NCH, NI], F32, space="PSUM", tag=f"pd{n}",
                         name=f"pd{n}") for n in range(NCH)]
        dots_chunk(1, pd1, 0)

        seg_reduce(0, seg0, psT0, *oh0)
        proj0, bc0 = epilogue(0, seg0, rec0)

        for c in range(1, 5):
            dots_chunk(1, pd1, c)

        outputs(0, proj0, bc0)

        psT1 = dance(1, pd1)
        seg_reduce(1, seg1, psT1, *oh1)
        proj1, bc1 = epilogue(1, seg1, rec1)
        outputs(1, proj1, bc1, final=True)

    nc.compile()
    return nc, names


def _get_module():
    if "mod" not in _CACHE:
        _CACHE["mod"] = _build_module()
    return _CACHE["mod"]


def _run(hidden, classifier_w, classifier_b, source_word_ids, target_word_ids,
         **spmd_kwargs):
    nc, names = _get_module()
    bf16 = ml_dtypes.bfloat16
    # [B, H, L], then pack row pairs so each pair tile is one contiguous
    # 8KB-per-partition DMA run: packed[b, t*128+p, k*L+l] = hidT[b, t*256+k*128+p, l]
    hidT = np.asarray(hidden).astype(bf16).transpose(0, 2, 1)
    hidT = np.ascontiguousarray(
        hidT.reshape(B, NH // 2, 2, P, L).transpose(0, 1, 3, 2, 4)
        .reshape(B, H // 2, 2 * L))
    w = np.asarray(classifier_w, dtype=np.float32).reshape(2 * H)
    # w2[hp, hq, s] = w_side[s][hq*128 + hp]  (device layout, no gather DMA)
    w2 = np.ascontiguousarray(
        np.stack([w[:H], w[H:]], axis=-1).reshape(NH, P, 2)
        .transpose(1, 0, 2).astype(bf16))
    bias = np.ascontiguousarray(
        np.broadcast_to(np.asarray(classifier_b, dtype=np.float32)
                        .reshape(1, 1), (P, 1)))
    src = np.ascontiguousarray(source_word_ids, dtype=np.int32)
    tgt = np.ascontiguousarray(target_word_ids, dtype=np.int32)

    in_maps = []
    for c in range(NCORES):
        m = {names["w"]: w2, names["b"]: bias}
        for b in range(BLOC):
            gb = c * BLOC + b
            m[names["hidT"][b]] = hidT[gb]
            m[names["src"][b]] = src[gb]
            m[names["tgt"][b]] = tgt[gb]
        in_maps.append(m)

    res = run_bass_kernel_spmd(nc, in_maps, core_ids=list(range(NCORES)),
                               **spmd_kwargs)
    out = np.empty((B, W, W, 1), dtype=np.float32)
    for c in range(NCORES):
        for b in range(BLOC):
            # device rows are (p, j)-major; un-permute during the upcast
            out[c * BLOC + b, :, :, 0] = (
                res.results[c][names["out"][b]].reshape(P, Q, W)
                .transpose(1, 0, 2).reshape(W, W).astype(np.float32))
    return out, res


def kernel(hidden, classifier_w, classifier_b, source_word_ids,
           target_word_ids, num_words):
    out, _ = _run(hidden, classifier_w, classifier_b, source_word_ids,
                  target_word_ids)
    return out
